# revision 1
# baseline (speedup 1.0000x reference)
"""Multi-head attention (B=2, S=2048, H=1024, NH=16, HD=64) on 8 TRN2 cores.

Sharding: tensor-parallel over heads - 2 heads per core. Each core:
  - projections in fp8-e4m3 hi/lo (exact 2-term split, 3 DoubleRow passes;
    weights pre-scaled by 2^5 to keep the lo residual out of fp8 subnormals)
  - scores transposed [k_pos(128), q(512)] per k-tile (bf16); exp on the
    scalar engine, ctx software-pipelined one k-tile-pair behind exp
  - ctx via lhsT = [v_h | 2^5] [128, 65]: out [65, 512] per head per bank;
    the scaled-ones column carries the softmax denominator as partition
    row 64 (no separate denominator matmuls) and exactly cancels the 2^5
    weight scale at normalization
  - normalization: bf16 reciprocal of the den row, broadcast across
    partitions by a ones-column matmul, per-head multiply; head1's ctx is
    partition-shifted to rows 64-127 by a small SBUF->SBUF DMA
  - per-block finalize (norm + out-proj) pipelined into the next block's
    first ktp slots; batch-1 projections interleaved as half-units
  - partial output  out_c = (ctx_c/den) @ Wo_c^T  [4096, 1024] in bf16.
Host sums the 8 partials and adds bo.
"""

import os
import numpy as np
import ml_dtypes

import concourse.bass as bass
import concourse.tile as tile
import concourse.mybir as mybir
from concourse import bacc
from concourse import bass_utils

F32 = mybir.dt.float32
BF16 = mybir.dt.bfloat16
NPBF16 = ml_dtypes.bfloat16

B = 2
S = 2048
H = 1024
NH = 16
HD = 64
NCORES = 8
HPC = NH // NCORES          # heads per core = 2
DSH = HPC * HD              # sharded feature dim per core = 128
ST = B * S                  # total tokens = 4096

NSB = ST // 512             # 8 s-blocks of 512 tokens
NKT_S = S // 128            # 16 k-tiles per batch in attention
NQB = S // 512              # 4 q-blocks per batch

OUTDT = os.environ.get("KOUTDT", "bf16")
CP_ENG = os.environ.get("KCP", "vector")   # engine for psum->sbuf copies
QKDT = os.environ.get("KQK", "bf16")       # f8 = e4m3 DoubleRow scores
PROJ = os.environ.get("KPROJ", "f8hl")     # f8hl = hi/lo e4m3 DoubleRow proj
F8 = mybir.dt.float8e4
NPF8 = ml_dtypes.float8_e4m3
WSCALE = 32.0 if PROJ == "f8hl" else 1.0   # 2^5: keeps w_lo out of fp8 subnormal range


def _build(n_kt: int, reps: int = 1):
    nc = bacc.Bacc("TRN2", target_bir_lowering=False, debug=False,
                   enable_asserts=True, num_devices=NCORES)

    odt = BF16 if OUTDT == "bf16" else F32
    if PROJ == "f8hl":
        ins = {}
        for nm in ("xhi", "xlo"):
            ins[nm] = nc.dram_tensor(nm, [n_kt * 128, ST], F8,
                                     kind="ExternalInput")
        for base in ("wq", "wk", "wv"):
            for sfx in ("hi", "lo"):
                nm = base + sfx
                ins[nm] = nc.dram_tensor(nm, [n_kt * 128, DSH], F8,
                                         kind="ExternalInput")
    else:
        ins = {"xT": nc.dram_tensor("xT", [n_kt * 128, ST], BF16,
                                    kind="ExternalInput")}
        for nm in ("wq", "wk", "wv"):
            ins[nm] = nc.dram_tensor(nm, [n_kt * 128, DSH], BF16,
                                     kind="ExternalInput")
    wo = nc.dram_tensor("wo", [DSH, H], BF16, kind="ExternalInput")
    out = nc.dram_tensor("out", [ST, H], odt, kind="ExternalOutput")
    ins = {k: t.ap() for k, t in ins.items()}
    wo, out = wo.ap(), out.ap()

    with tile.TileContext(nc) as tc:
        for _ in range(reps):
            _emit(tc, n_kt, ins, wo, out, odt)
    nc.compile()
    return nc


def _emit(tc, n_kt, ins, wo, out, odt):
    nc = tc.nc
    cp = getattr(nc, CP_ENG)
    ctx_pools = []

    def pool(name, bufs, space="SBUF"):
        p = tc.alloc_tile_pool(name=name, bufs=bufs, space=space)
        ctx_pools.append(p)
        return p

    # --- pools -----------------------------------------------------------
    xp = pool("x", n_kt * NSB // 2)            # x tiles [128, 1024] bf16
    pw = pool("w", n_kt)                       # weight tiles [128, 128] bf16
    pwo = pool("wo", 2)                        # [128, 1024] bf16
    pqk = pool("qk", 4)                        # qT/kT [128, 4096] bf16/f8
    pqk8 = pool("qk8", 2) if QKDT == "f8" else None  # folded [32,2,2,ST] f8
    pv = pool("v", 2)                          # v_aug [128, 32, 2, 65] bf16
    pexp = pool("exp", 6)                      # expT [128, 1024] bf16
    prec = pool("rec", 2)                      # rden/rb [128, 2, 512] f32
    pcq = pool("cq", 3)                        # ctxT [128, 512] bf16
    pc1 = pool("c1", 4)                        # ctx h1 staging [128,512] bf16
    pout = pool("outsb", 4)                    # out staging [128, 1024] odt
    # PSUM: scores 2x2 banks + ctx 2x1 + outproj 2x1 = 8 banks
    PP = pool("pp", 2, space="PSUM")           # scores [128, 1024] f32
    PC = pool("pc", 2, space="PSUM")           # ctx [65, 512] f32 (1 bank)
    PO = pool("po", 2, space="PSUM")           # outproj [128, 512] f32

    # --- load weights and x (priority order, alternating DMA queues) ----
    wo_t = pwo.tile([128, H], BF16)
    NP = n_kt // 2                             # DoubleRow kt-pairs
    LFT = n_kt % 2                             # leftover kt (bias row block)
    dmas = {"q": [], "x0": [], "kv": [], "x": [], "last": [(wo_t[:], wo[:, :])]}

    if PROJ == "f8hl":
        xhi, xlo = ins["xhi"], ins["xlo"]
        # x pair tiles [128, 2, 1024] per (hi/lo, ktp, sbp)
        x8 = {s: [[None] * (NSB // 2) for _ in range(NP)] for s in "hl"}
        xL = [None] * (NSB // 2)               # leftover (bias) [128, 2, 1024]
        for sbp in range(NSB // 2):
            cols = slice(sbp * 1024, (sbp + 1) * 1024)
            for ktp in range(NP):
                rows = slice(ktp * 256, (ktp + 1) * 256)
                for s, src in (("h", xhi), ("l", xlo)):
                    t = xp.tile([128, 2, 1024], F8, name="x")
                    x8[s][ktp][sbp] = t
                    dmas["x0" if sbp == 0 else "x"].append(
                        (t[:], src[rows, cols]
                         .rearrange("(i p) c -> p i c", p=128)))
            if LFT:
                t = xp.tile([128, 2, 1024], F8, name="xL", bufs=NSB // 2)
                xL[sbp] = t
                rows = slice(NP * 256, NP * 256 + 128)
                dmas["x0" if sbp == 0 else "x"].append(
                    (t[:, 0, :], xhi[rows, cols]))
                dmas["x0" if sbp == 0 else "x"].append(
                    (t[:, 1, :], xhi[rows, cols]))
        # weight pair tiles [128, 2, 128] per (hi/lo, ktp) + leftover pair
        w8 = {}
        for base, key in (("wq", "q"), ("wk", "kv"), ("wv", "kv")):
            for s, sfx in (("h", "hi"), ("l", "lo")):
                lst = []
                for ktp in range(NP):
                    t = pw.tile([128, 2, DSH], F8, name=base)
                    rows = slice(ktp * 256, (ktp + 1) * 256)
                    dmas[key].append((t[:], ins[base + sfx][rows, :]
                                      .rearrange("(i p) c -> p i c", p=128)))
                    lst.append(t)
                w8[base, s] = lst
            if LFT:
                t = pw.tile([128, 2, DSH], F8, name=base + "L", bufs=1)
                rows = slice(NP * 256, NP * 256 + 128)
                dmas[key].append((t[:, 0, :], ins[base + "hi"][rows, :]))
                dmas[key].append((t[:, 1, :], ins[base + "lo"][rows, :]))
                w8[base, "L"] = t
    else:
        xT, wq, wk, wv = ins["xT"], ins["wq"], ins["wk"], ins["wv"]
        wq_t, wk_t, wv_t = [], [], []
        for kt in range(n_kt):
            for lst, nm in ((wq_t, "wq"), (wk_t, "wk"), (wv_t, "wv")):
                t = pw.tile([128, DSH], BF16, name=nm)
                lst.append(t)
        x_t = [[None] * (NSB // 2) for _ in range(n_kt)]
        for sbp in range(NSB // 2):
            for kt in range(n_kt):
                x_t[kt][sbp] = xp.tile([128, 1024], BF16, name="x")
        for kt in range(n_kt):
            dmas["q"].append((wq_t[kt][:], wq[kt * 128:(kt + 1) * 128, :]))
            dmas["x0"].append((x_t[kt][0][:],
                               xT[kt * 128:(kt + 1) * 128, 0:1024]))
            dmas["kv"].append((wk_t[kt][:], wk[kt * 128:(kt + 1) * 128, :]))
            dmas["kv"].append((wv_t[kt][:], wv[kt * 128:(kt + 1) * 128, :]))
        for sbp in range(1, NSB // 2):
            for kt in range(n_kt):
                dmas["x"].append((x_t[kt][sbp][:],
                                  xT[kt * 128:(kt + 1) * 128,
                                     sbp * 1024:(sbp + 1) * 1024]))

    # ACT is the steady-state bottleneck: keep its queue free of input DMAs
    for i, (dst, src) in enumerate(dmas["q"] + dmas["x0"]):
        (nc.sync if i % 2 == 0 else nc.gpsimd).dma_start(dst, src)
    for i, (dst, src) in enumerate(dmas["kv"] + dmas["x"] + dmas["last"]):
        (nc.gpsimd if i % 2 == 0 else nc.sync).dma_start(dst, src)

    qkdt = F8 if QKDT == "f8" else BF16
    qT = pqk.tile([128, ST], qkdt, tag="qk")
    kT = pqk.tile([128, ST], qkdt, tag="qk")
    if QKDT == "f8":
        # folded layout for DoubleRow: [32 p, 2 head, 2 dhalf, s]
        qTf = pqk8.tile([32, HPC, 2, ST], F8, tag="qk8")
        kTf = pqk8.tile([32, HPC, 2, ST], F8, tag="qk8")
    # v_aug [128 kpos, 32 gtile, 2 head, 65]; col 64 = ones (from memset)
    v_sb = pv.tile([128, ST // 128, HPC, HD + 1], BF16)
    nc.gpsimd.memset(v_sb[:], WSCALE)
    ones_t = pwo.tile([128, 128], BF16, tag="ones")
    nc.gpsimd.memset(ones_t[:], 1.0)

    # --- projection units (one PSUM tile fill + copy each) --------------
    DR = mybir.MatmulPerfMode.DoubleRow
    PASSES = (("h", "h"), ("h", "l"), ("l", "h"))

    # qT/kT: out[d(128), s] ; lhsT = w [h,d], rhs = x [h,s]; one 512-col half
    def proj_qk(base, w_list, dst, dstf, sbp, half, pp):
        ps = pp.tile([128, 512], F32, tag="pp" if pp is PP else "po",
                     name="psp")
        hs = slice(half * 512, (half + 1) * 512)
        if PROJ == "f8hl":
            seq = [(w8[base, a][ktp][:], x8[b2][ktp][sbp][:, :, hs])
                   for a, b2 in PASSES for ktp in range(NP)]
            if LFT:
                seq.append((w8[base, "L"][:], xL[sbp][:, :, hs]))
            for i, (lh, rh) in enumerate(seq):
                nc.tensor.matmul(ps[:], lh, rh, start=(i == 0),
                                 stop=(i == len(seq) - 1), perf_mode=DR)
        else:
            for kt in range(n_kt):
                nc.tensor.matmul(ps[:], w_list[kt][:],
                                 x_t[kt][sbp][:, hs],
                                 start=(kt == 0), stop=(kt == n_kt - 1))
        cols = slice(sbp * 1024 + half * 512, sbp * 1024 + (half + 1) * 512)
        cp.tensor_copy(dst[:, cols], ps[:])
        if QKDT == "f8":                       # fold quarters into [32,2,2,s]
            for h in range(HPC):
                for dh in range(2):
                    r0 = h * 64 + dh * 32
                    nc.sync.dma_start(dstf[:, h, dh, cols],
                                      dst[r0:r0 + 32, cols])

    # v: out[s(128), (h d)] ; lhsT = x slice [h, s128], rhs = wv
    def proj_v(sbp, half, pp):
        ps = pp.tile([128, 512], F32, tag="pp" if pp is PP else "po",
                     name="psp")
        for ssb in range(4):
            po_ = slice(ssb * 128, (ssb + 1) * 128)
            xo_ = slice(half * 512 + ssb * 128, half * 512 + (ssb + 1) * 128)
            if PROJ == "f8hl":
                seq = [(x8[a][ktp][sbp][:, :, xo_], w8["wv", b2][ktp][:])
                       for a, b2 in PASSES for ktp in range(NP)]
                if LFT:
                    seq.append((xL[sbp][:, :, xo_], w8["wv", "L"][:]))
                for i, (lh, rh) in enumerate(seq):
                    nc.tensor.matmul(ps[:, po_], lh, rh, start=(i == 0),
                                     stop=(i == len(seq) - 1), perf_mode=DR)
            else:
                for kt in range(n_kt):
                    nc.tensor.matmul(ps[:, po_], x_t[kt][sbp][:, xo_],
                                     wv_t[kt][:],
                                     start=(kt == 0), stop=(kt == n_kt - 1))
        # scatter ps cols (g4, h, d) into v slots; col HD stays WSCALE (den)
        pv_in = ps[:].rearrange("p (g h d) -> p g h d", h=HPC, d=HD)
        g0 = sbp * 8 + half * 4
        for h in range(HPC):
            cp.tensor_copy(v_sb[:, g0:g0 + 4, h, 0:HD], pv_in[:, :, h, :])

    def proj_unit(kind, sbp, half, pp):
        if kind == "q":
            proj_qk("wq", None if PROJ == "f8hl" else wq_t, qT,
                    qTf if QKDT == "f8" else None, sbp, half, pp)
        elif kind == "k":
            proj_qk("wk", None if PROJ == "f8hl" else wk_t, kT,
                    kTf if QKDT == "f8" else None, sbp, half, pp)
        else:
            proj_v(sbp, half, pp)

    # batch-0 projections up front; batch-1 interleaved as small half-units
    # (through the PO psum pool) so the ACT-bound loop is only mildly starved
    for sbp in range(2):
        for kind in ("q", "k", "v"):
            for half in range(2):
                proj_unit(kind, sbp, half, PP)
    deferred = [(kind, sbp, half) for sbp in (2, 3) for kind in ("q", "k", "v")
                for half in range(2)]

    # --- attention + out-projection, per (batch, q-block) ---------------
    def finalize(q0, cps):
        # normalize: rec = 1/den (bf16, row 64); broadcast via ones-matmul
        rden = prec.tile([128, 2, 512], BF16, tag="rden")
        rbs = prec.tile([128, 2, 512], BF16, tag="rbs")
        cq = pcq.tile([128, 512], BF16, tag="cq")
        c1 = pc1.tile([128, 512], BF16, tag="c1")
        for h in range(2):
            with nc.allow_low_precision(reason="bf16 recip feeds matmul"):
                nc.vector.reciprocal(rden[64:65, h, :], cps[h][64:65, :])
            rbh = PO.tile([128, 512], F32, tag="po", name="rb")
            nc.tensor.matmul(rbh[:], ones_t[64:65, :], rden[64:65, h, :],
                             start=True, stop=True)
            cp.tensor_copy(rbs[0:64, h, :], rbh[0:64, :])
        nc.vector.tensor_mul(cq[0:64, :], cps[0][0:64, :], rbs[0:64, 0, :])
        nc.vector.tensor_mul(c1[0:64, :], cps[1][0:64, :], rbs[0:64, 1, :])
        nc.sync.dma_start(cq[64:128, :], c1[0:64, :])
        return cq

    def finalize_proj(q0, cq, ssbs):
        # out rows q0..q0+512 : lhsT = cq col-slice, rhs = wo
        for ssb in ssbs:
            c0 = q0 + ssb * 128
            ot = pout.tile([128, H], odt, tag="ot")
            for e in range(2):
                po = PO.tile([128, 512], F32, tag="po")
                nc.tensor.matmul(po[:], cq[:, ssb * 128:(ssb + 1) * 128],
                                 wo_t[:, e * 512:(e + 1) * 512],
                                 start=True, stop=True)
                cp.tensor_copy(ot[:, e * 512:(e + 1) * 512], po[:])
            nc.sync.dma_start(out[c0:c0 + 128, :], ot[:])

    def emit_ctx(cps, b, ktp, expt):
        for j in range(2):                     # ctx+den accumulate
            kt = ktp * 2 + j
            g = b * NKT_S + kt
            st = (ktp == 0 and j == 0)
            sp = (ktp == NKT_S // 2 - 1 and j == 1)
            for h in range(2):
                nc.tensor.matmul(cps[h][:, :],
                                 v_sb[:, g, h, :],
                                 expt[h][:, j * 512:(j + 1) * 512],
                                 start=st, stop=sp)

    pending = None                             # (q0, cps) awaiting finalize
    lag = None                                 # (cps, b, ktp, expt) ctx lag
    for b in range(B):
        for qb in range(NQB):
            q0 = b * S + qb * 512              # global column of this q-block
            # per-head psum bank: ctx rows 0-63, den row 64
            cps = [PC.tile([65, 512], F32, tag="pc", name=f"cps{h}")
                   for h in range(2)]
            for ktp in range(NKT_S // 2):
                expt = []
                for h in range(2):             # scores, row-packed pairs
                    sc = PP.tile([128, 1024], F32, tag="pp")
                    for j in range(2):
                        kt = ktp * 2 + j
                        k0 = b * S + kt * 128
                        if QKDT == "f8":
                            nc.tensor.matmul(
                                sc[:, j * 512:(j + 1) * 512],
                                kTf[:, h, :, k0:k0 + 128],
                                qTf[:, h, :, q0:q0 + 512],
                                start=True, stop=True,
                                perf_mode=mybir.MatmulPerfMode.DoubleRow)
                        else:
                            nc.tensor.matmul(
                                sc[:, j * 512:(j + 1) * 512],
                                kT[h * 64:(h + 1) * 64, k0:k0 + 128],
                                qT[h * 64:(h + 1) * 64, q0:q0 + 512],
                                start=True, stop=True)
                    e = pexp.tile([128, 1024], BF16)
                    nc.scalar.activation(e[:], sc[:],
                                         mybir.ActivationFunctionType.Exp,
                                         scale=0.125 / (WSCALE * WSCALE))
                    expt.append(e)
                # ctx lags one ktp behind scores/exp so PE never waits on
                # ACT; the lagged last-ktp ctx spills past the next block's
                # first scores, closing the block-boundary gap
                if lag is not None:
                    emit_ctx(*lag)
                lag = (cps, b, ktp, expt)
                if ktp == 0 and pending is not None:
                    pending_cq = finalize(*pending)
                elif ktp == 1 and pending is not None:
                    finalize_proj(pending[0], pending_cq, (0, 1))
                elif ktp == 2 and pending is not None:
                    finalize_proj(pending[0], pending_cq, (2, 3))
                    pending = None
                elif deferred:
                    proj_unit(*deferred.pop(0), PO)
            pending = (q0, cps)
    emit_ctx(*lag)
    finalize_proj(pending[0], finalize(*pending), (0, 1, 2, 3))

    for p in reversed(ctx_pools):
        p.release()


_CACHE = {}


def _get_nc(n_kt):
    if n_kt not in _CACHE:
        _CACHE[n_kt] = _build(n_kt)
    return _CACHE[n_kt]


def _prep_inputs(hidden_states, Wq, bq, Wk, bk, Wv, bv, Wo, bo):
    x = np.ascontiguousarray(np.asarray(hidden_states, np.float32)
                             .reshape(ST, H))
    bias = not (np.all(bq == 0) and np.all(bk == 0) and np.all(bv == 0))
    n_kt = 9 if bias else 8
    xTn = np.zeros((n_kt * 128, ST), np.float32)
    xTn[:H] = x.T
    if bias:
        xTn[H] = 1.0
    
    if PROJ == "f8hl":
        xhi = xTn.astype(NPF8)
        xm = {"xhi": xhi,
              "xlo": (xTn - xhi.astype(np.float32)).astype(NPF8)}
    else:
        xm = {"xT": xTn.astype(NPBF16)}

    in_maps = []
    for c in range(NCORES):
        rows = slice(c * DSH, (c + 1) * DSH)
        m = dict(xm)
        for name, W, bvec in (("wq", Wq, bq), ("wk", Wk, bk), ("wv", Wv, bv)):
            wt = np.zeros((n_kt * 128, DSH), np.float32)
            wt[:H] = np.asarray(W, np.float32)[rows, :].T
            if bias:
                wt[H] = np.asarray(bvec, np.float32)[rows]
            wt *= WSCALE
            if PROJ == "f8hl":
                whi = wt.astype(NPF8)
                m[name + "hi"] = whi
                m[name + "lo"] = (wt - whi.astype(np.float32)).astype(NPF8)
            else:
                m[name] = wt.astype(NPBF16)
        m["wo"] = np.ascontiguousarray(
            np.asarray(Wo, np.float32)[:, rows].T).astype(NPBF16)
        in_maps.append(m)
    return n_kt, in_maps


def kernel(hidden_states, Wq, bq, Wk, bk, Wv, bv, Wo, bo, _return_extras=False):
    n_kt, in_maps = _prep_inputs(hidden_states, Wq, bq, Wk, bk, Wv, bv, Wo, bo)
    nc = _get_nc(n_kt)
    res = bass_utils.run_bass_kernel_spmd(nc, in_maps,
                                          core_ids=list(range(NCORES)))
    acc = res.results[0]["out"].astype(np.float64)
    for c in range(1, NCORES):
        acc += res.results[c]["out"]
    acc += np.asarray(bo, np.float64)
    outv = acc.astype(np.float32).reshape(B, S, H)
    if _return_extras:
        return outv, (nc, in_maps, res)
    return outv



# revision 4
# speedup vs baseline: 3.5793x; 3.5793x over previous
"""Multi-head attention (B=2, S=2048, H=1024, NH=16, HD=64) on 8 TRN2 cores.

Sharding: tensor-parallel over heads - 2 heads per core. Each core:
  - projections in fp8-e4m3 hi/lo (exact 2-term split, 3 DoubleRow passes;
    weights pre-scaled by 2^5 to keep the lo residual out of fp8 subnormals)
  - scores transposed [k_pos(128), q(512)] per k-tile (bf16); exp on the
    scalar engine, ctx software-pipelined one k-tile-pair behind exp
  - ctx via lhsT = [v_h | 2^5] [128, 65]: out [65, 512] per head per bank;
    the scaled-ones column carries the softmax denominator as partition
    row 64 (no separate denominator matmuls) and exactly cancels the 2^5
    weight scale at normalization
  - normalization: bf16 reciprocal of the den row, broadcast across
    partitions by a ones-column matmul, per-head multiply; head1's ctx is
    partition-shifted to rows 64-127 by a small SBUF->SBUF DMA
  - per-block finalize (norm + out-proj) pipelined into the next block's
    first ktp slots; batch-1 projections interleaved as half-units
  - partial output  out_c = (ctx_c/den) @ Wo_c^T  [4096, 1024] in bf16.
Host sums the 8 partials and adds bo.
"""

import os
import numpy as np
import ml_dtypes

import concourse.bass as bass
import concourse.tile as tile
import concourse.mybir as mybir
from concourse import bacc
from concourse import bass_utils

F32 = mybir.dt.float32
BF16 = mybir.dt.bfloat16
NPBF16 = ml_dtypes.bfloat16

B = 2
S = 2048
H = 1024
NH = 16
HD = 64
NCORES = 8
HPC = NH // NCORES          # heads per core = 2
DSH = HPC * HD              # sharded feature dim per core = 128
ST = B * S                  # total tokens = 4096

NSB = ST // 512             # 8 s-blocks of 512 tokens
NKT_S = S // 128            # 16 k-tiles per batch in attention
NQB = S // 512              # 4 q-blocks per batch

OUTDT = os.environ.get("KOUTDT", "bf16")
CP_ENG = os.environ.get("KCP", "vector")   # engine for psum->sbuf copies
QKDT = os.environ.get("KQK", "bf16")       # f8 = e4m3 DoubleRow scores
PROJ = os.environ.get("KPROJ", "f8hl")     # f8hl = hi/lo e4m3 DoubleRow proj
F8 = mybir.dt.float8e4
NPF8 = ml_dtypes.float8_e4m3
WSCALE = 32.0 if PROJ == "f8hl" else 1.0   # 2^5: keeps w_lo out of fp8 subnormal range


def _build(n_kt: int, reps: int = 1):
    nc = bacc.Bacc("TRN2", target_bir_lowering=False, debug=False,
                   enable_asserts=True, num_devices=NCORES)

    odt = BF16 if OUTDT == "bf16" else F32
    if PROJ == "f8hl":
        ins = {}
        for nm in ("xhi", "xlo"):
            ins[nm] = nc.dram_tensor(nm, [n_kt * 128, ST], F8,
                                     kind="ExternalInput")
        for base in ("wq", "wk", "wv"):
            for sfx in ("hi", "lo"):
                nm = base + sfx
                ins[nm] = nc.dram_tensor(nm, [n_kt * 128, DSH], F8,
                                         kind="ExternalInput")
    else:
        ins = {"xT": nc.dram_tensor("xT", [n_kt * 128, ST], BF16,
                                    kind="ExternalInput")}
        for nm in ("wq", "wk", "wv"):
            ins[nm] = nc.dram_tensor(nm, [n_kt * 128, DSH], BF16,
                                     kind="ExternalInput")
    wo = nc.dram_tensor("wo", [DSH, H], BF16, kind="ExternalInput")
    out = nc.dram_tensor("out", [ST, H], odt, kind="ExternalOutput")
    ins = {k: t.ap() for k, t in ins.items()}
    wo, out = wo.ap(), out.ap()

    with tile.TileContext(nc) as tc:
        # pools live across reps so consecutive bodies pipeline on device
        pools = {}
        pool_order = []

        def pool(name, bufs, space="SBUF"):
            p = tc.alloc_tile_pool(name=name, bufs=bufs, space=space)
            pools[name] = p
            pool_order.append(p)
            return p

        pool("x", n_kt * NSB // 2)             # x tiles [128, 1024] bf16
        pool("w", n_kt)                        # weight tiles [128, 128] bf16
        pool("wo", 2)                          # [128, 1024] bf16
        pool("qk", 4)                          # qT/kT [128, 4096] bf16/f8
        if QKDT == "f8":
            pool("qk8", 2)                     # folded [32,2,2,ST] f8
        pool("v", 2)                           # v_aug [128, 32, 2, 65] bf16
        pool("exp", 6)                         # expT [128, 1024] bf16
        pool("rec", 2)                         # rden/rb [128, 2, 512] f32
        pool("cq", 3)                          # ctxT [128, 512] bf16
        pool("c1", 4)                          # ctx h1 staging [128,512] bf16
        pool("outsb", 4)                       # out staging [128, 1024] odt
        # PSUM: scores 2x2 banks + ctx 2x1 + outproj 2x1 = 8 banks
        pool("pp", 2, space="PSUM")            # scores [128, 1024] f32
        pool("pc", 2, space="PSUM")            # ctx [65, 512] f32 (1 bank)
        pool("po", 2, space="PSUM")            # outproj [128, 512] f32

        for _ in range(reps):
            _emit(tc, n_kt, ins, wo, out, odt, pools)
        for p in reversed(pool_order):
            p.release()
    nc.compile()
    return nc


def _emit(tc, n_kt, ins, wo, out, odt, pools):
    nc = tc.nc
    cp = getattr(nc, CP_ENG)

    xp = pools["x"]
    pw = pools["w"]
    pwo = pools["wo"]
    pqk = pools["qk"]
    pqk8 = pools.get("qk8")
    pv = pools["v"]
    pexp = pools["exp"]
    prec = pools["rec"]
    pcq = pools["cq"]
    pc1 = pools["c1"]
    pout = pools["outsb"]
    PP = pools["pp"]
    PC = pools["pc"]
    PO = pools["po"]

    # --- load weights and x (priority order, alternating DMA queues) ----
    wo_t = pwo.tile([128, H], BF16)
    NP = n_kt // 2                             # DoubleRow kt-pairs
    LFT = n_kt % 2                             # leftover kt (bias row block)
    dmas = {"q": [], "x0": [], "kv": [], "x": [], "last": [(wo_t[:], wo[:, :])]}

    if PROJ == "f8hl":
        xhi, xlo = ins["xhi"], ins["xlo"]
        # x pair tiles [128, 2, 1024] per (hi/lo, ktp, sbp)
        x8 = {s: [[None] * (NSB // 2) for _ in range(NP)] for s in "hl"}
        xL = [None] * (NSB // 2)               # leftover (bias) [128, 2, 1024]
        for sbp in range(NSB // 2):
            cols = slice(sbp * 1024, (sbp + 1) * 1024)
            for ktp in range(NP):
                rows = slice(ktp * 256, (ktp + 1) * 256)
                for s, src in (("h", xhi), ("l", xlo)):
                    t = xp.tile([128, 2, 1024], F8, name="x")
                    x8[s][ktp][sbp] = t
                    dmas["x0" if sbp == 0 else "x"].append(
                        (t[:], src[rows, cols]
                         .rearrange("(i p) c -> p i c", p=128)))
            if LFT:
                t = xp.tile([128, 2, 1024], F8, name="xL", bufs=NSB // 2)
                xL[sbp] = t
                rows = slice(NP * 256, NP * 256 + 128)
                dmas["x0" if sbp == 0 else "x"].append(
                    (t[:, 0, :], xhi[rows, cols]))
                dmas["x0" if sbp == 0 else "x"].append(
                    (t[:, 1, :], xhi[rows, cols]))
        # weight pair tiles [128, 2, 128] per (hi/lo, ktp) + leftover pair
        w8 = {}
        for base, key in (("wq", "q"), ("wk", "kv"), ("wv", "kv")):
            for s, sfx in (("h", "hi"), ("l", "lo")):
                lst = []
                for ktp in range(NP):
                    t = pw.tile([128, 2, DSH], F8, name=base)
                    rows = slice(ktp * 256, (ktp + 1) * 256)
                    dmas[key].append((t[:], ins[base + sfx][rows, :]
                                      .rearrange("(i p) c -> p i c", p=128)))
                    lst.append(t)
                w8[base, s] = lst
            if LFT:
                t = pw.tile([128, 2, DSH], F8, name=base + "L", bufs=1)
                rows = slice(NP * 256, NP * 256 + 128)
                dmas[key].append((t[:, 0, :], ins[base + "hi"][rows, :]))
                dmas[key].append((t[:, 1, :], ins[base + "lo"][rows, :]))
                w8[base, "L"] = t
    else:
        xT, wq, wk, wv = ins["xT"], ins["wq"], ins["wk"], ins["wv"]
        wq_t, wk_t, wv_t = [], [], []
        for kt in range(n_kt):
            for lst, nm in ((wq_t, "wq"), (wk_t, "wk"), (wv_t, "wv")):
                t = pw.tile([128, DSH], BF16, name=nm)
                lst.append(t)
        x_t = [[None] * (NSB // 2) for _ in range(n_kt)]
        for sbp in range(NSB // 2):
            for kt in range(n_kt):
                x_t[kt][sbp] = xp.tile([128, 1024], BF16, name="x")
        for kt in range(n_kt):
            dmas["q"].append((wq_t[kt][:], wq[kt * 128:(kt + 1) * 128, :]))
            dmas["x0"].append((x_t[kt][0][:],
                               xT[kt * 128:(kt + 1) * 128, 0:1024]))
            dmas["kv"].append((wk_t[kt][:], wk[kt * 128:(kt + 1) * 128, :]))
            dmas["kv"].append((wv_t[kt][:], wv[kt * 128:(kt + 1) * 128, :]))
        for sbp in range(1, NSB // 2):
            for kt in range(n_kt):
                dmas["x"].append((x_t[kt][sbp][:],
                                  xT[kt * 128:(kt + 1) * 128,
                                     sbp * 1024:(sbp + 1) * 1024]))

    # ACT is the steady-state bottleneck: keep its queue free of input DMAs
    for i, (dst, src) in enumerate(dmas["q"] + dmas["x0"]):
        (nc.sync if i % 2 == 0 else nc.gpsimd).dma_start(dst, src)
    for i, (dst, src) in enumerate(dmas["kv"] + dmas["x"] + dmas["last"]):
        (nc.gpsimd if i % 2 == 0 else nc.sync).dma_start(dst, src)

    qkdt = F8 if QKDT == "f8" else BF16
    qT = pqk.tile([128, ST], qkdt, tag="qk")
    kT = pqk.tile([128, ST], qkdt, tag="qk")
    if QKDT == "f8":
        # folded layout for DoubleRow: [32 p, 2 head, 2 dhalf, s]
        qTf = pqk8.tile([32, HPC, 2, ST], F8, tag="qk8")
        kTf = pqk8.tile([32, HPC, 2, ST], F8, tag="qk8")
    # v_aug [128 kpos, 32 gtile, 2 head, 65]; col 64 = ones (from memset)
    v_sb = pv.tile([128, ST // 128, HPC, HD + 1], BF16)
    nc.gpsimd.memset(v_sb[:], WSCALE)
    ones_t = pwo.tile([128, 128], BF16, tag="ones")
    nc.gpsimd.memset(ones_t[:], 1.0)

    # --- projection units (one PSUM tile fill + copy each) --------------
    DR = mybir.MatmulPerfMode.DoubleRow
    PASSES = (("h", "h"), ("h", "l"), ("l", "h"))

    # qT/kT: out[d(128), s] ; lhsT = w [h,d], rhs = x [h,s]; one 512-col half
    def proj_qk(base, w_list, dst, dstf, sbp, half, pp):
        ps = pp.tile([128, 512], F32, tag="pp" if pp is PP else "po",
                     name="psp")
        hs = slice(half * 512, (half + 1) * 512)
        if PROJ == "f8hl":
            seq = [(w8[base, a][ktp][:], x8[b2][ktp][sbp][:, :, hs])
                   for a, b2 in PASSES for ktp in range(NP)]
            if LFT:
                seq.append((w8[base, "L"][:], xL[sbp][:, :, hs]))
            for i, (lh, rh) in enumerate(seq):
                nc.tensor.matmul(ps[:], lh, rh, start=(i == 0),
                                 stop=(i == len(seq) - 1), perf_mode=DR)
        else:
            for kt in range(n_kt):
                nc.tensor.matmul(ps[:], w_list[kt][:],
                                 x_t[kt][sbp][:, hs],
                                 start=(kt == 0), stop=(kt == n_kt - 1))
        cols = slice(sbp * 1024 + half * 512, sbp * 1024 + (half + 1) * 512)
        cp.tensor_copy(dst[:, cols], ps[:])
        if QKDT == "f8":                       # fold quarters into [32,2,2,s]
            for h in range(HPC):
                for dh in range(2):
                    r0 = h * 64 + dh * 32
                    nc.sync.dma_start(dstf[:, h, dh, cols],
                                      dst[r0:r0 + 32, cols])

    # v: out[s(128), (h d)] ; lhsT = x slice [h, s128], rhs = wv
    def proj_v(sbp, half, pp):
        ps = pp.tile([128, 512], F32, tag="pp" if pp is PP else "po",
                     name="psp")
        for ssb in range(4):
            po_ = slice(ssb * 128, (ssb + 1) * 128)
            xo_ = slice(half * 512 + ssb * 128, half * 512 + (ssb + 1) * 128)
            if PROJ == "f8hl":
                seq = [(x8[a][ktp][sbp][:, :, xo_], w8["wv", b2][ktp][:])
                       for a, b2 in PASSES for ktp in range(NP)]
                if LFT:
                    seq.append((xL[sbp][:, :, xo_], w8["wv", "L"][:]))
                for i, (lh, rh) in enumerate(seq):
                    nc.tensor.matmul(ps[:, po_], lh, rh, start=(i == 0),
                                     stop=(i == len(seq) - 1), perf_mode=DR)
            else:
                for kt in range(n_kt):
                    nc.tensor.matmul(ps[:, po_], x_t[kt][sbp][:, xo_],
                                     wv_t[kt][:],
                                     start=(kt == 0), stop=(kt == n_kt - 1))
        # scatter ps cols (g4, h, d) into v slots; col HD stays WSCALE (den)
        pv_in = ps[:].rearrange("p (g h d) -> p g h d", h=HPC, d=HD)
        g0 = sbp * 8 + half * 4
        for h in range(HPC):
            cp.tensor_copy(v_sb[:, g0:g0 + 4, h, 0:HD], pv_in[:, :, h, :])

    def proj_unit(kind, sbp, half, pp):
        if kind == "q":
            proj_qk("wq", None if PROJ == "f8hl" else wq_t, qT,
                    qTf if QKDT == "f8" else None, sbp, half, pp)
        elif kind == "k":
            proj_qk("wk", None if PROJ == "f8hl" else wk_t, kT,
                    kTf if QKDT == "f8" else None, sbp, half, pp)
        else:
            proj_v(sbp, half, pp)

    # batch-0 projections up front; batch-1 interleaved as small half-units
    # (through the PO psum pool) so the ACT-bound loop is only mildly starved
    for sbp in range(2):
        for kind in ("q", "k", "v"):
            for half in range(2):
                proj_unit(kind, sbp, half, PP)
    deferred = [(kind, sbp, half) for sbp in (2, 3) for kind in ("q", "k", "v")
                for half in range(2)]

    # --- attention + out-projection, per (batch, q-block) ---------------
    def finalize(q0, cps):
        # normalize: rec = 1/den (bf16, row 64); broadcast via ones-matmul
        rden = prec.tile([128, 2, 512], BF16, tag="rden")
        rbs = prec.tile([128, 2, 512], BF16, tag="rbs")
        cq = pcq.tile([128, 512], BF16, tag="cq")
        c1 = pc1.tile([128, 512], BF16, tag="c1")
        for h in range(2):
            with nc.allow_low_precision(reason="bf16 recip feeds matmul"):
                nc.vector.reciprocal(rden[64:65, h, :], cps[h][64:65, :])
            rbh = PO.tile([128, 512], F32, tag="po", name="rb")
            nc.tensor.matmul(rbh[:], ones_t[64:65, :], rden[64:65, h, :],
                             start=True, stop=True)
            cp.tensor_copy(rbs[0:64, h, :], rbh[0:64, :])
        nc.vector.tensor_mul(cq[0:64, :], cps[0][0:64, :], rbs[0:64, 0, :])
        nc.vector.tensor_mul(c1[0:64, :], cps[1][0:64, :], rbs[0:64, 1, :])
        nc.sync.dma_start(cq[64:128, :], c1[0:64, :])
        return cq

    def finalize_proj(q0, cq, ssbs):
        # out rows q0..q0+512 : lhsT = cq col-slice, rhs = wo
        for ssb in ssbs:
            c0 = q0 + ssb * 128
            ot = pout.tile([128, H], odt, tag="ot")
            for e in range(2):
                po = PO.tile([128, 512], F32, tag="po")
                nc.tensor.matmul(po[:], cq[:, ssb * 128:(ssb + 1) * 128],
                                 wo_t[:, e * 512:(e + 1) * 512],
                                 start=True, stop=True)
                cp.tensor_copy(ot[:, e * 512:(e + 1) * 512], po[:])
            nc.sync.dma_start(out[c0:c0 + 128, :], ot[:])

    def emit_ctx(cps, b, ktp, expt):
        for j in range(2):                     # ctx+den accumulate
            kt = ktp * 2 + j
            g = b * NKT_S + kt
            st = (ktp == 0 and j == 0)
            sp = (ktp == NKT_S // 2 - 1 and j == 1)
            for h in range(2):
                nc.tensor.matmul(cps[h][:, :],
                                 v_sb[:, g, h, :],
                                 expt[h][:, j * 512:(j + 1) * 512],
                                 start=st, stop=sp)

    pending = None                             # (q0, cps) awaiting finalize
    lag = None                                 # (cps, b, ktp, expt) ctx lag
    for b in range(B):
        for qb in range(NQB):
            q0 = b * S + qb * 512              # global column of this q-block
            # per-head psum bank: ctx rows 0-63, den row 64
            cps = [PC.tile([65, 512], F32, tag="pc", name=f"cps{h}")
                   for h in range(2)]
            for ktp in range(NKT_S // 2):
                expt = []
                for h in range(2):             # scores, row-packed pairs
                    sc = PP.tile([128, 1024], F32, tag="pp")
                    for j in range(2):
                        kt = ktp * 2 + j
                        k0 = b * S + kt * 128
                        if QKDT == "f8":
                            nc.tensor.matmul(
                                sc[:, j * 512:(j + 1) * 512],
                                kTf[:, h, :, k0:k0 + 128],
                                qTf[:, h, :, q0:q0 + 512],
                                start=True, stop=True,
                                perf_mode=mybir.MatmulPerfMode.DoubleRow)
                        else:
                            nc.tensor.matmul(
                                sc[:, j * 512:(j + 1) * 512],
                                kT[h * 64:(h + 1) * 64, k0:k0 + 128],
                                qT[h * 64:(h + 1) * 64, q0:q0 + 512],
                                start=True, stop=True)
                    e = pexp.tile([128, 1024], BF16)
                    nc.scalar.activation(e[:], sc[:],
                                         mybir.ActivationFunctionType.Exp,
                                         scale=0.125 / (WSCALE * WSCALE))
                    expt.append(e)
                # ctx lags one ktp behind scores/exp so PE never waits on
                # ACT; the lagged last-ktp ctx spills past the next block's
                # first scores, closing the block-boundary gap
                if lag is not None:
                    emit_ctx(*lag)
                lag = (cps, b, ktp, expt)
                if ktp == 0 and pending is not None:
                    pending_cq = finalize(*pending)
                elif ktp == 1 and pending is not None:
                    finalize_proj(pending[0], pending_cq, (0, 1))
                elif ktp == 2 and pending is not None:
                    finalize_proj(pending[0], pending_cq, (2, 3))
                    pending = None
                elif deferred:
                    proj_unit(*deferred.pop(0), PO)
            pending = (q0, cps)
    emit_ctx(*lag)
    finalize_proj(pending[0], finalize(*pending), (0, 1, 2, 3))


_CACHE = {}


def _get_nc(n_kt):
    if n_kt not in _CACHE:
        _CACHE[n_kt] = _build(n_kt)
    return _CACHE[n_kt]


def _prep_inputs(hidden_states, Wq, bq, Wk, bk, Wv, bv, Wo, bo):
    x = np.ascontiguousarray(np.asarray(hidden_states, np.float32)
                             .reshape(ST, H))
    bias = not (np.all(bq == 0) and np.all(bk == 0) and np.all(bv == 0))
    n_kt = 9 if bias else 8
    xTn = np.zeros((n_kt * 128, ST), np.float32)
    xTn[:H] = x.T
    if bias:
        xTn[H] = 1.0
    
    if PROJ == "f8hl":
        xhi = xTn.astype(NPF8)
        xm = {"xhi": xhi,
              "xlo": (xTn - xhi.astype(np.float32)).astype(NPF8)}
    else:
        xm = {"xT": xTn.astype(NPBF16)}

    in_maps = []
    for c in range(NCORES):
        rows = slice(c * DSH, (c + 1) * DSH)
        m = dict(xm)
        for name, W, bvec in (("wq", Wq, bq), ("wk", Wk, bk), ("wv", Wv, bv)):
            wt = np.zeros((n_kt * 128, DSH), np.float32)
            wt[:H] = np.asarray(W, np.float32)[rows, :].T
            if bias:
                wt[H] = np.asarray(bvec, np.float32)[rows]
            wt *= WSCALE
            if PROJ == "f8hl":
                whi = wt.astype(NPF8)
                m[name + "hi"] = whi
                m[name + "lo"] = (wt - whi.astype(np.float32)).astype(NPF8)
            else:
                m[name] = wt.astype(NPBF16)
        m["wo"] = np.ascontiguousarray(
            np.asarray(Wo, np.float32)[:, rows].T).astype(NPBF16)
        in_maps.append(m)
    return n_kt, in_maps


def kernel(hidden_states, Wq, bq, Wk, bk, Wv, bv, Wo, bo, _return_extras=False):
    n_kt, in_maps = _prep_inputs(hidden_states, Wq, bq, Wk, bk, Wv, bv, Wo, bo)
    nc = _get_nc(n_kt)
    res = bass_utils.run_bass_kernel_spmd(nc, in_maps,
                                          core_ids=list(range(NCORES)))
    acc = res.results[0]["out"].astype(np.float64)
    for c in range(1, NCORES):
        acc += res.results[c]["out"]
    acc += np.asarray(bo, np.float64)
    outv = acc.astype(np.float32).reshape(B, S, H)
    if _return_extras:
        return outv, (nc, in_maps, res)
    return outv



# revision 6
# speedup vs baseline: 4.3836x; 1.2247x over previous
"""Multi-head attention (B=2, S=2048, H=1024, NH=16, HD=64) on 8 TRN2 cores.

Sharding: tensor-parallel over heads - 2 heads per core. Each core:
  - projections in bf16 (KPROJ=f8hl selects the exact fp8-e4m3 hi/lo
    2-term split, 3 DoubleRow passes, weights pre-scaled by 2^5; slower
    on HW than one bf16 pass)
  - scores transposed [k_pos(128), q(512)] per k-tile (bf16); exp on the
    scalar engine, ctx software-pipelined one k-tile-pair behind exp
  - ctx via lhsT = [v_h | 2^5] [128, 65]: out [65, 512] per head per bank;
    the scaled-ones column carries the softmax denominator as partition
    row 64 (no separate denominator matmuls) and exactly cancels the 2^5
    weight scale at normalization
  - normalization: bf16 reciprocal of the den row, broadcast across
    partitions by a ones-column matmul, per-head multiply; head1's ctx is
    partition-shifted to rows 64-127 by a small SBUF->SBUF DMA
  - per-block finalize (norm + out-proj) pipelined into the next block's
    first ktp slots; batch-1 projections interleaved as half-units
  - partial output  out_c = (ctx_c/den) @ Wo_c^T  [4096, 1024] in bf16.
Host sums the 8 partials and adds bo.
"""

import os
import numpy as np
import ml_dtypes

import concourse.bass as bass
import concourse.tile as tile
import concourse.mybir as mybir
from concourse import bacc
from concourse import bass_utils

F32 = mybir.dt.float32
BF16 = mybir.dt.bfloat16
NPBF16 = ml_dtypes.bfloat16

B = 2
S = 2048
H = 1024
NH = 16
HD = 64
NCORES = 8
HPC = NH // NCORES          # heads per core = 2
DSH = HPC * HD              # sharded feature dim per core = 128
ST = B * S                  # total tokens = 4096

NSB = ST // 512             # 8 s-blocks of 512 tokens
NKT_S = S // 128            # 16 k-tiles per batch in attention
NQB = S // 512              # 4 q-blocks per batch

OUTDT = os.environ.get("KOUTDT", "bf16")
CP_ENG = os.environ.get("KCP", "vector")   # engine for psum->sbuf copies
QKDT = os.environ.get("KQK", "bf16")       # f8 = e4m3 DoubleRow scores
# bf16 beats the hi/lo-fp8 DoubleRow path on HW for the projections: DR is
# ~1.9x bf16 per contraction row, but the exact hi/lo split needs 3 passes
# (1.5x the rows), netting ~1.6x more PE time than one bf16 pass.
PROJ = os.environ.get("KPROJ", "bf16")     # f8hl = hi/lo e4m3 DoubleRow proj
F8 = mybir.dt.float8e4
NPF8 = ml_dtypes.float8_e4m3
WSCALE = 32.0 if PROJ == "f8hl" else 1.0   # 2^5: keeps w_lo out of fp8 subnormal range


def _build(n_kt: int, reps: int = 1):
    nc = bacc.Bacc("TRN2", target_bir_lowering=False, debug=False,
                   enable_asserts=True, num_devices=NCORES)

    odt = BF16 if OUTDT == "bf16" else F32
    if PROJ == "f8hl":
        ins = {}
        for nm in ("xhi", "xlo"):
            ins[nm] = nc.dram_tensor(nm, [n_kt * 128, ST], F8,
                                     kind="ExternalInput")
        for base in ("wq", "wk", "wv"):
            for sfx in ("hi", "lo"):
                nm = base + sfx
                ins[nm] = nc.dram_tensor(nm, [n_kt * 128, DSH], F8,
                                         kind="ExternalInput")
    else:
        ins = {"xT": nc.dram_tensor("xT", [n_kt * 128, ST], BF16,
                                    kind="ExternalInput")}
        for nm in ("wq", "wk", "wv"):
            ins[nm] = nc.dram_tensor(nm, [n_kt * 128, DSH], BF16,
                                     kind="ExternalInput")
    wo = nc.dram_tensor("wo", [DSH, H], BF16, kind="ExternalInput")
    out = nc.dram_tensor("out", [ST, H], odt, kind="ExternalOutput")
    ins = {k: t.ap() for k, t in ins.items()}
    wo, out = wo.ap(), out.ap()

    with tile.TileContext(nc) as tc:
        # pools live across reps so consecutive bodies pipeline on device
        pools = {}
        pool_order = []

        def pool(name, bufs, space="SBUF"):
            p = tc.alloc_tile_pool(name=name, bufs=bufs, space=space)
            pools[name] = p
            pool_order.append(p)
            return p

        pool("x", n_kt * NSB // 2)             # x tiles [128, 1024] bf16
        pool("w", n_kt)                        # weight tiles [128, 128] bf16
        pool("wo", 2)                          # [128, 1024] bf16
        pool("qk", 4)                          # qT/kT [128, 4096] bf16/f8
        if QKDT == "f8":
            pool("qk8", 2)                     # folded [32,2,2,ST] f8
        pool("v", 2)                           # v_aug [128, 32, 2, 65] bf16
        pool("exp", 6)                         # expT [128, 1024] bf16
        pool("rec", 2)                         # rden/rb [128, 2, 512] f32
        pool("cq", 3)                          # ctxT [128, 512] bf16
        pool("c1", 4)                          # ctx h1 staging [128,512] bf16
        pool("outsb", 4)                       # out staging [128, 1024] odt
        # PSUM: scores 2x2 banks + ctx 2x1 + outproj 2x1 = 8 banks
        pool("pp", 2, space="PSUM")            # scores [128, 1024] f32
        pool("pc", 2, space="PSUM")            # ctx [65, 512] f32 (1 bank)
        pool("po", 2, space="PSUM")            # outproj [128, 512] f32

        for _ in range(reps):
            _emit(tc, n_kt, ins, wo, out, odt, pools)
        for p in reversed(pool_order):
            p.release()
    nc.compile()
    return nc


def _emit(tc, n_kt, ins, wo, out, odt, pools):
    nc = tc.nc
    cp = getattr(nc, CP_ENG)

    xp = pools["x"]
    pw = pools["w"]
    pwo = pools["wo"]
    pqk = pools["qk"]
    pqk8 = pools.get("qk8")
    pv = pools["v"]
    pexp = pools["exp"]
    prec = pools["rec"]
    pcq = pools["cq"]
    pc1 = pools["c1"]
    pout = pools["outsb"]
    PP = pools["pp"]
    PC = pools["pc"]
    PO = pools["po"]

    # --- load weights and x (priority order, alternating DMA queues) ----
    wo_t = pwo.tile([128, H], BF16)
    NP = n_kt // 2                             # DoubleRow kt-pairs
    LFT = n_kt % 2                             # leftover kt (bias row block)
    dmas = {"q": [], "x0": [], "kv": [], "x": [], "last": [(wo_t[:], wo[:, :])]}

    if PROJ == "f8hl":
        xhi, xlo = ins["xhi"], ins["xlo"]
        # x pair tiles [128, 2, 1024] per (hi/lo, ktp, sbp)
        x8 = {s: [[None] * (NSB // 2) for _ in range(NP)] for s in "hl"}
        xL = [None] * (NSB // 2)               # leftover (bias) [128, 2, 1024]
        for sbp in range(NSB // 2):
            cols = slice(sbp * 1024, (sbp + 1) * 1024)
            for ktp in range(NP):
                rows = slice(ktp * 256, (ktp + 1) * 256)
                for s, src in (("h", xhi), ("l", xlo)):
                    t = xp.tile([128, 2, 1024], F8, name="x")
                    x8[s][ktp][sbp] = t
                    dmas["x0" if sbp == 0 else "x"].append(
                        (t[:], src[rows, cols]
                         .rearrange("(i p) c -> p i c", p=128)))
            if LFT:
                t = xp.tile([128, 2, 1024], F8, name="xL", bufs=NSB // 2)
                xL[sbp] = t
                rows = slice(NP * 256, NP * 256 + 128)
                dmas["x0" if sbp == 0 else "x"].append(
                    (t[:, 0, :], xhi[rows, cols]))
                dmas["x0" if sbp == 0 else "x"].append(
                    (t[:, 1, :], xhi[rows, cols]))
        # weight pair tiles [128, 2, 128] per (hi/lo, ktp) + leftover pair
        w8 = {}
        for base, key in (("wq", "q"), ("wk", "kv"), ("wv", "kv")):
            for s, sfx in (("h", "hi"), ("l", "lo")):
                lst = []
                for ktp in range(NP):
                    t = pw.tile([128, 2, DSH], F8, name=base)
                    rows = slice(ktp * 256, (ktp + 1) * 256)
                    dmas[key].append((t[:], ins[base + sfx][rows, :]
                                      .rearrange("(i p) c -> p i c", p=128)))
                    lst.append(t)
                w8[base, s] = lst
            if LFT:
                t = pw.tile([128, 2, DSH], F8, name=base + "L", bufs=1)
                rows = slice(NP * 256, NP * 256 + 128)
                dmas[key].append((t[:, 0, :], ins[base + "hi"][rows, :]))
                dmas[key].append((t[:, 1, :], ins[base + "lo"][rows, :]))
                w8[base, "L"] = t
    else:
        xT, wq, wk, wv = ins["xT"], ins["wq"], ins["wk"], ins["wv"]
        wq_t, wk_t, wv_t = [], [], []
        for kt in range(n_kt):
            for lst, nm in ((wq_t, "wq"), (wk_t, "wk"), (wv_t, "wv")):
                t = pw.tile([128, DSH], BF16, name=nm)
                lst.append(t)
        x_t = [[None] * (NSB // 2) for _ in range(n_kt)]
        for sbp in range(NSB // 2):
            for kt in range(n_kt):
                x_t[kt][sbp] = xp.tile([128, 1024], BF16, name="x")
        for kt in range(n_kt):
            dmas["q"].append((wq_t[kt][:], wq[kt * 128:(kt + 1) * 128, :]))
            dmas["x0"].append((x_t[kt][0][:],
                               xT[kt * 128:(kt + 1) * 128, 0:1024]))
            dmas["kv"].append((wk_t[kt][:], wk[kt * 128:(kt + 1) * 128, :]))
            dmas["kv"].append((wv_t[kt][:], wv[kt * 128:(kt + 1) * 128, :]))
        for sbp in range(1, NSB // 2):
            for kt in range(n_kt):
                dmas["x"].append((x_t[kt][sbp][:],
                                  xT[kt * 128:(kt + 1) * 128,
                                     sbp * 1024:(sbp + 1) * 1024]))

    # ACT is the steady-state bottleneck: keep its queue free of input DMAs
    for i, (dst, src) in enumerate(dmas["q"] + dmas["x0"]):
        (nc.sync if i % 2 == 0 else nc.gpsimd).dma_start(dst, src)
    for i, (dst, src) in enumerate(dmas["kv"] + dmas["x"] + dmas["last"]):
        (nc.gpsimd if i % 2 == 0 else nc.sync).dma_start(dst, src)

    qkdt = F8 if QKDT == "f8" else BF16
    qT = pqk.tile([128, ST], qkdt, tag="qk")
    kT = pqk.tile([128, ST], qkdt, tag="qk")
    if QKDT == "f8":
        # folded layout for DoubleRow: [32 p, 2 head, 2 dhalf, s]
        qTf = pqk8.tile([32, HPC, 2, ST], F8, tag="qk8")
        kTf = pqk8.tile([32, HPC, 2, ST], F8, tag="qk8")
    # v_aug [128 kpos, 32 gtile, 2 head, 65]; col 64 = ones (from memset)
    v_sb = pv.tile([128, ST // 128, HPC, HD + 1], BF16)
    nc.gpsimd.memset(v_sb[:], WSCALE)
    ones_t = pwo.tile([128, 128], BF16, tag="ones")
    nc.gpsimd.memset(ones_t[:], 1.0)

    # --- projection units (one PSUM tile fill + copy each) --------------
    DR = mybir.MatmulPerfMode.DoubleRow
    PASSES = (("h", "h"), ("h", "l"), ("l", "h"))

    # qT/kT: out[d(128), s] ; lhsT = w [h,d], rhs = x [h,s]; one 512-col half
    def proj_qk(base, w_list, dst, dstf, sbp, half, pp):
        ps = pp.tile([128, 512], F32, tag="pp" if pp is PP else "po",
                     name="psp")
        hs = slice(half * 512, (half + 1) * 512)
        if PROJ == "f8hl":
            seq = [(w8[base, a][ktp][:], x8[b2][ktp][sbp][:, :, hs])
                   for a, b2 in PASSES for ktp in range(NP)]
            if LFT:
                seq.append((w8[base, "L"][:], xL[sbp][:, :, hs]))
            for i, (lh, rh) in enumerate(seq):
                nc.tensor.matmul(ps[:], lh, rh, start=(i == 0),
                                 stop=(i == len(seq) - 1), perf_mode=DR)
        else:
            for kt in range(n_kt):
                nc.tensor.matmul(ps[:], w_list[kt][:],
                                 x_t[kt][sbp][:, hs],
                                 start=(kt == 0), stop=(kt == n_kt - 1))
        cols = slice(sbp * 1024 + half * 512, sbp * 1024 + (half + 1) * 512)
        cp.tensor_copy(dst[:, cols], ps[:])
        if QKDT == "f8":                       # fold quarters into [32,2,2,s]
            for h in range(HPC):
                for dh in range(2):
                    r0 = h * 64 + dh * 32
                    nc.sync.dma_start(dstf[:, h, dh, cols],
                                      dst[r0:r0 + 32, cols])

    # v: out[s(128), (h d)] ; lhsT = x slice [h, s128], rhs = wv
    def proj_v(sbp, half, pp):
        ps = pp.tile([128, 512], F32, tag="pp" if pp is PP else "po",
                     name="psp")
        for ssb in range(4):
            po_ = slice(ssb * 128, (ssb + 1) * 128)
            xo_ = slice(half * 512 + ssb * 128, half * 512 + (ssb + 1) * 128)
            if PROJ == "f8hl":
                seq = [(x8[a][ktp][sbp][:, :, xo_], w8["wv", b2][ktp][:])
                       for a, b2 in PASSES for ktp in range(NP)]
                if LFT:
                    seq.append((xL[sbp][:, :, xo_], w8["wv", "L"][:]))
                for i, (lh, rh) in enumerate(seq):
                    nc.tensor.matmul(ps[:, po_], lh, rh, start=(i == 0),
                                     stop=(i == len(seq) - 1), perf_mode=DR)
            else:
                for kt in range(n_kt):
                    nc.tensor.matmul(ps[:, po_], x_t[kt][sbp][:, xo_],
                                     wv_t[kt][:],
                                     start=(kt == 0), stop=(kt == n_kt - 1))
        # scatter ps cols (g4, h, d) into v slots; col HD stays WSCALE (den)
        pv_in = ps[:].rearrange("p (g h d) -> p g h d", h=HPC, d=HD)
        g0 = sbp * 8 + half * 4
        for h in range(HPC):
            cp.tensor_copy(v_sb[:, g0:g0 + 4, h, 0:HD], pv_in[:, :, h, :])

    def proj_unit(kind, sbp, half, pp):
        if kind == "q":
            proj_qk("wq", None if PROJ == "f8hl" else wq_t, qT,
                    qTf if QKDT == "f8" else None, sbp, half, pp)
        elif kind == "k":
            proj_qk("wk", None if PROJ == "f8hl" else wk_t, kT,
                    kTf if QKDT == "f8" else None, sbp, half, pp)
        else:
            proj_v(sbp, half, pp)

    # batch-0 projections up front; batch-1 interleaved as small half-units
    # (through the PO psum pool) so the ACT-bound loop is only mildly starved
    for sbp in range(2):
        for kind in ("q", "k", "v"):
            for half in range(2):
                proj_unit(kind, sbp, half, PP)
    deferred = [(kind, sbp, half) for sbp in (2, 3) for kind in ("q", "k", "v")
                for half in range(2)]

    # --- attention + out-projection, per (batch, q-block) ---------------
    def finalize(q0, cps):
        # normalize: rec = 1/den (bf16, row 64); broadcast via ones-matmul
        rden = prec.tile([128, 2, 512], BF16, tag="rden")
        rbs = prec.tile([128, 2, 512], BF16, tag="rbs")
        cq = pcq.tile([128, 512], BF16, tag="cq")
        c1 = pc1.tile([128, 512], BF16, tag="c1")
        for h in range(2):
            with nc.allow_low_precision(reason="bf16 recip feeds matmul"):
                nc.vector.reciprocal(rden[64:65, h, :], cps[h][64:65, :])
            rbh = PO.tile([128, 512], F32, tag="po", name="rb")
            nc.tensor.matmul(rbh[:], ones_t[64:65, :], rden[64:65, h, :],
                             start=True, stop=True)
            cp.tensor_copy(rbs[0:64, h, :], rbh[0:64, :])
        nc.vector.tensor_mul(cq[0:64, :], cps[0][0:64, :], rbs[0:64, 0, :])
        nc.vector.tensor_mul(c1[0:64, :], cps[1][0:64, :], rbs[0:64, 1, :])
        nc.sync.dma_start(cq[64:128, :], c1[0:64, :])
        return cq

    def finalize_proj(q0, cq, ssbs):
        # out rows q0..q0+512 : lhsT = cq col-slice, rhs = wo
        for ssb in ssbs:
            c0 = q0 + ssb * 128
            ot = pout.tile([128, H], odt, tag="ot")
            for e in range(2):
                po = PO.tile([128, 512], F32, tag="po")
                nc.tensor.matmul(po[:], cq[:, ssb * 128:(ssb + 1) * 128],
                                 wo_t[:, e * 512:(e + 1) * 512],
                                 start=True, stop=True)
                cp.tensor_copy(ot[:, e * 512:(e + 1) * 512], po[:])
            nc.sync.dma_start(out[c0:c0 + 128, :], ot[:])

    def emit_ctx(cps, b, ktp, expt):
        for j in range(2):                     # ctx+den accumulate
            kt = ktp * 2 + j
            g = b * NKT_S + kt
            st = (ktp == 0 and j == 0)
            sp = (ktp == NKT_S // 2 - 1 and j == 1)
            for h in range(2):
                nc.tensor.matmul(cps[h][:, :],
                                 v_sb[:, g, h, :],
                                 expt[h][:, j * 512:(j + 1) * 512],
                                 start=st, stop=sp)

    pending = None                             # (q0, cps) awaiting finalize
    lag = None                                 # (cps, b, ktp, expt) ctx lag
    for b in range(B):
        for qb in range(NQB):
            q0 = b * S + qb * 512              # global column of this q-block
            # per-head psum bank: ctx rows 0-63, den row 64
            cps = [PC.tile([65, 512], F32, tag="pc", name=f"cps{h}")
                   for h in range(2)]
            for ktp in range(NKT_S // 2):
                expt = []
                for h in range(2):             # scores, row-packed pairs
                    sc = PP.tile([128, 1024], F32, tag="pp")
                    for j in range(2):
                        kt = ktp * 2 + j
                        k0 = b * S + kt * 128
                        if QKDT == "f8":
                            nc.tensor.matmul(
                                sc[:, j * 512:(j + 1) * 512],
                                kTf[:, h, :, k0:k0 + 128],
                                qTf[:, h, :, q0:q0 + 512],
                                start=True, stop=True,
                                perf_mode=mybir.MatmulPerfMode.DoubleRow)
                        else:
                            nc.tensor.matmul(
                                sc[:, j * 512:(j + 1) * 512],
                                kT[h * 64:(h + 1) * 64, k0:k0 + 128],
                                qT[h * 64:(h + 1) * 64, q0:q0 + 512],
                                start=True, stop=True)
                    e = pexp.tile([128, 1024], BF16)
                    nc.scalar.activation(e[:], sc[:],
                                         mybir.ActivationFunctionType.Exp,
                                         scale=0.125 / (WSCALE * WSCALE))
                    expt.append(e)
                # ctx lags one ktp behind scores/exp so PE never waits on
                # ACT; the lagged last-ktp ctx spills past the next block's
                # first scores, closing the block-boundary gap
                if lag is not None:
                    emit_ctx(*lag)
                lag = (cps, b, ktp, expt)
                if ktp == 0 and pending is not None:
                    pending_cq = finalize(*pending)
                elif ktp == 1 and pending is not None:
                    finalize_proj(pending[0], pending_cq, (0, 1))
                elif ktp == 2 and pending is not None:
                    finalize_proj(pending[0], pending_cq, (2, 3))
                    pending = None
                elif deferred:
                    proj_unit(*deferred.pop(0), PO)
            pending = (q0, cps)
    emit_ctx(*lag)
    finalize_proj(pending[0], finalize(*pending), (0, 1, 2, 3))


_CACHE = {}


def _get_nc(n_kt):
    if n_kt not in _CACHE:
        _CACHE[n_kt] = _build(n_kt)
    return _CACHE[n_kt]


def _prep_inputs(hidden_states, Wq, bq, Wk, bk, Wv, bv, Wo, bo):
    x = np.ascontiguousarray(np.asarray(hidden_states, np.float32)
                             .reshape(ST, H))
    bias = not (np.all(bq == 0) and np.all(bk == 0) and np.all(bv == 0))
    n_kt = 9 if bias else 8
    xTn = np.zeros((n_kt * 128, ST), np.float32)
    xTn[:H] = x.T
    if bias:
        xTn[H] = 1.0
    
    if PROJ == "f8hl":
        xhi = xTn.astype(NPF8)
        xm = {"xhi": xhi,
              "xlo": (xTn - xhi.astype(np.float32)).astype(NPF8)}
    else:
        xm = {"xT": xTn.astype(NPBF16)}

    in_maps = []
    for c in range(NCORES):
        rows = slice(c * DSH, (c + 1) * DSH)
        m = dict(xm)
        for name, W, bvec in (("wq", Wq, bq), ("wk", Wk, bk), ("wv", Wv, bv)):
            wt = np.zeros((n_kt * 128, DSH), np.float32)
            wt[:H] = np.asarray(W, np.float32)[rows, :].T
            if bias:
                wt[H] = np.asarray(bvec, np.float32)[rows]
            wt *= WSCALE
            if PROJ == "f8hl":
                whi = wt.astype(NPF8)
                m[name + "hi"] = whi
                m[name + "lo"] = (wt - whi.astype(np.float32)).astype(NPF8)
            else:
                m[name] = wt.astype(NPBF16)
        m["wo"] = np.ascontiguousarray(
            np.asarray(Wo, np.float32)[:, rows].T).astype(NPBF16)
        in_maps.append(m)
    return n_kt, in_maps


def kernel(hidden_states, Wq, bq, Wk, bk, Wv, bv, Wo, bo, _return_extras=False):
    n_kt, in_maps = _prep_inputs(hidden_states, Wq, bq, Wk, bk, Wv, bv, Wo, bo)
    nc = _get_nc(n_kt)
    res = bass_utils.run_bass_kernel_spmd(nc, in_maps,
                                          core_ids=list(range(NCORES)))
    acc = res.results[0]["out"].astype(np.float64)
    for c in range(1, NCORES):
        acc += res.results[c]["out"]
    acc += np.asarray(bo, np.float64)
    outv = acc.astype(np.float32).reshape(B, S, H)
    if _return_extras:
        return outv, (nc, in_maps, res)
    return outv



# revision 7
# speedup vs baseline: 5.4013x; 1.2322x over previous
"""Multi-head attention (B=2, S=2048, H=1024, NH=16, HD=64) on 8 TRN2 cores.

Sharding: tensor-parallel over heads - 2 heads per core. Each core:
  - projections in bf16 (KPROJ=f8hl selects the exact fp8-e4m3 hi/lo
    2-term split, 3 DoubleRow passes, weights pre-scaled by 2^5; slower
    on HW than one bf16 pass)
  - scores transposed [k_pos(128), q(512)] per k-tile (bf16); exp on the
    scalar engine, ctx software-pipelined one k-tile-pair behind exp
  - ctx via lhsT = [v_h | 2^5] [128, 65]: out [65, 512] per head per bank;
    the scaled-ones column carries the softmax denominator as partition
    row 64 (no separate denominator matmuls) and exactly cancels the 2^5
    weight scale at normalization
  - normalization: bf16 reciprocal of the den row, broadcast across
    partitions by a ones-column matmul, per-head multiply; head1's ctx is
    partition-shifted to rows 64-127 by a small SBUF->SBUF DMA
  - per-block finalize (norm + out-proj) pipelined into the next block's
    first ktp slots; batch-1 projections interleaved as half-units
  - partial output  out_c = (ctx_c/den) @ Wo_c^T  [4096, 1024] in bf16.
Host sums the 8 partials and adds bo.
"""

import os
import numpy as np
import ml_dtypes

import concourse.bass as bass
import concourse.tile as tile
import concourse.mybir as mybir
from concourse import bacc
from concourse import bass_utils

F32 = mybir.dt.float32
BF16 = mybir.dt.bfloat16
NPBF16 = ml_dtypes.bfloat16

B = 2
S = 2048
H = 1024
NH = 16
HD = 64
NCORES = 8
HPC = NH // NCORES          # heads per core = 2
DSH = HPC * HD              # sharded feature dim per core = 128
ST = B * S                  # total tokens = 4096

NSB = ST // 512             # 8 s-blocks of 512 tokens
NKT_S = S // 128            # 16 k-tiles per batch in attention
NQB = S // 512              # 4 q-blocks per batch

OUTDT = os.environ.get("KOUTDT", "bf16")
CP_ENG = os.environ.get("KCP", "vector")   # engine for psum->sbuf copies
QKDT = os.environ.get("KQK", "bf16")       # f8 = e4m3 DoubleRow scores
# bf16 beats the hi/lo-fp8 DoubleRow path on HW for the projections: DR is
# ~1.9x bf16 per contraction row, but the exact hi/lo split needs 3 passes
# (1.5x the rows), netting ~1.6x more PE time than one bf16 pass.
PROJ = os.environ.get("KPROJ", "bf16")     # f8hl = hi/lo e4m3 DoubleRow proj
F8 = mybir.dt.float8e4
NPF8 = ml_dtypes.float8_e4m3
WSCALE = 32.0 if PROJ == "f8hl" else 1.0   # 2^5: keeps w_lo out of fp8 subnormal range


def _build(n_kt: int, reps: int = 1):
    nc = bacc.Bacc("TRN2", target_bir_lowering=False, debug=False,
                   enable_asserts=True, num_devices=NCORES)

    odt = BF16 if OUTDT == "bf16" else F32
    if PROJ == "f8hl":
        ins = {}
        for nm in ("xhi", "xlo"):
            ins[nm] = nc.dram_tensor(nm, [n_kt * 128, ST], F8,
                                     kind="ExternalInput")
        for base in ("wq", "wk", "wv"):
            for sfx in ("hi", "lo"):
                nm = base + sfx
                ins[nm] = nc.dram_tensor(nm, [n_kt * 128, DSH], F8,
                                         kind="ExternalInput")
    else:
        ins = {"xT": nc.dram_tensor("xT", [n_kt * 128, ST], BF16,
                                    kind="ExternalInput")}
        for nm in ("wq", "wk", "wv"):
            ins[nm] = nc.dram_tensor(nm, [n_kt * 128, DSH], BF16,
                                     kind="ExternalInput")
    wo = nc.dram_tensor("wo", [DSH, H], BF16, kind="ExternalInput")
    out = nc.dram_tensor("out", [ST, H], odt, kind="ExternalOutput")
    ins = {k: t.ap() for k, t in ins.items()}
    wo, out = wo.ap(), out.ap()

    with tile.TileContext(nc) as tc:
        # pools live across reps so consecutive bodies pipeline on device
        pools = {}
        pool_order = []

        def pool(name, bufs, space="SBUF"):
            p = tc.alloc_tile_pool(name=name, bufs=bufs, space=space)
            pools[name] = p
            pool_order.append(p)
            return p

        pool("x", n_kt * NSB // 2)             # x tiles [128, 1024] bf16
        pool("w", n_kt)                        # weight tiles [128, 128] bf16
        pool("wo", 2)                          # [128, 1024] bf16
        pool("qk", 4)                          # qT/kT [128, 4096] bf16/f8
        if QKDT == "f8":
            pool("qk8", 2)                     # folded [32,2,2,ST] f8
        pool("v", 2)                           # v_aug [128, 32, 2, 65] bf16
        pool("exp", 6)                         # expT [128, 1024] bf16
        pool("rec", 2)                         # rden/rb [128, 2, 512] f32
        pool("cq", 3)                          # ctxT [128, 512] bf16
        pool("c1", 4)                          # ctx h1 staging [128,512] bf16
        pool("outsb", 4)                       # out staging [128, 1024] odt
        # PSUM: scores 2x2 banks + ctx 2x1 + outproj 2x1 = 8 banks
        pool("pp", 2, space="PSUM")            # scores [128, 1024] f32
        pool("pc", 2, space="PSUM")            # ctx [65, 512] f32 (1 bank)
        pool("po", 2, space="PSUM")            # outproj [128, 512] f32

        for _ in range(reps):
            _emit(tc, n_kt, ins, wo, out, odt, pools)
        for p in reversed(pool_order):
            p.release()
    nc.compile()
    return nc


def _emit(tc, n_kt, ins, wo, out, odt, pools):
    nc = tc.nc
    cp = getattr(nc, CP_ENG)

    xp = pools["x"]
    pw = pools["w"]
    pwo = pools["wo"]
    pqk = pools["qk"]
    pqk8 = pools.get("qk8")
    pv = pools["v"]
    pexp = pools["exp"]
    prec = pools["rec"]
    pcq = pools["cq"]
    pc1 = pools["c1"]
    pout = pools["outsb"]
    PP = pools["pp"]
    PC = pools["pc"]
    PO = pools["po"]

    # --- load weights and x (priority order, alternating DMA queues) ----
    wo_t = pwo.tile([128, H], BF16)
    NP = n_kt // 2                             # DoubleRow kt-pairs
    LFT = n_kt % 2                             # leftover kt (bias row block)
    dmas = {"q": [], "x0": [], "kv": [], "x": [], "last": [(wo_t[:], wo[:, :])]}

    if PROJ == "f8hl":
        xhi, xlo = ins["xhi"], ins["xlo"]
        # x pair tiles [128, 2, 1024] per (hi/lo, ktp, sbp)
        x8 = {s: [[None] * (NSB // 2) for _ in range(NP)] for s in "hl"}
        xL = [None] * (NSB // 2)               # leftover (bias) [128, 2, 1024]
        for sbp in range(NSB // 2):
            cols = slice(sbp * 1024, (sbp + 1) * 1024)
            for ktp in range(NP):
                rows = slice(ktp * 256, (ktp + 1) * 256)
                for s, src in (("h", xhi), ("l", xlo)):
                    t = xp.tile([128, 2, 1024], F8, name="x")
                    x8[s][ktp][sbp] = t
                    dmas["x0" if sbp == 0 else "x"].append(
                        (t[:], src[rows, cols]
                         .rearrange("(i p) c -> p i c", p=128)))
            if LFT:
                t = xp.tile([128, 2, 1024], F8, name="xL", bufs=NSB // 2)
                xL[sbp] = t
                rows = slice(NP * 256, NP * 256 + 128)
                dmas["x0" if sbp == 0 else "x"].append(
                    (t[:, 0, :], xhi[rows, cols]))
                dmas["x0" if sbp == 0 else "x"].append(
                    (t[:, 1, :], xhi[rows, cols]))
        # weight pair tiles [128, 2, 128] per (hi/lo, ktp) + leftover pair
        w8 = {}
        for base, key in (("wq", "q"), ("wk", "kv"), ("wv", "kv")):
            for s, sfx in (("h", "hi"), ("l", "lo")):
                lst = []
                for ktp in range(NP):
                    t = pw.tile([128, 2, DSH], F8, name=base)
                    rows = slice(ktp * 256, (ktp + 1) * 256)
                    dmas[key].append((t[:], ins[base + sfx][rows, :]
                                      .rearrange("(i p) c -> p i c", p=128)))
                    lst.append(t)
                w8[base, s] = lst
            if LFT:
                t = pw.tile([128, 2, DSH], F8, name=base + "L", bufs=1)
                rows = slice(NP * 256, NP * 256 + 128)
                dmas[key].append((t[:, 0, :], ins[base + "hi"][rows, :]))
                dmas[key].append((t[:, 1, :], ins[base + "lo"][rows, :]))
                w8[base, "L"] = t
    else:
        xT, wq, wk, wv = ins["xT"], ins["wq"], ins["wk"], ins["wv"]
        wq_t, wk_t, wv_t = [], [], []
        for kt in range(n_kt):
            for lst, nm in ((wq_t, "wq"), (wk_t, "wk"), (wv_t, "wv")):
                t = pw.tile([128, DSH], BF16, name=nm)
                lst.append(t)
        x_t = [[None] * (NSB // 2) for _ in range(n_kt)]
        for sbp in range(NSB // 2):
            for kt in range(n_kt):
                x_t[kt][sbp] = xp.tile([128, 1024], BF16, name="x")
        for kt in range(n_kt):
            dmas["q"].append((wq_t[kt][:], wq[kt * 128:(kt + 1) * 128, :]))
            dmas["x0"].append((x_t[kt][0][:],
                               xT[kt * 128:(kt + 1) * 128, 0:1024]))
            dmas["kv"].append((wk_t[kt][:], wk[kt * 128:(kt + 1) * 128, :]))
            dmas["kv"].append((wv_t[kt][:], wv[kt * 128:(kt + 1) * 128, :]))
        for sbp in range(1, NSB // 2):
            for kt in range(n_kt):
                dmas["x"].append((x_t[kt][sbp][:],
                                  xT[kt * 128:(kt + 1) * 128,
                                     sbp * 1024:(sbp + 1) * 1024]))

    # ACT is the steady-state bottleneck: keep its queue free of input DMAs
    for i, (dst, src) in enumerate(dmas["q"] + dmas["x0"]):
        (nc.sync if i % 2 == 0 else nc.gpsimd).dma_start(dst, src)
    for i, (dst, src) in enumerate(dmas["kv"] + dmas["x"] + dmas["last"]):
        (nc.gpsimd if i % 2 == 0 else nc.sync).dma_start(dst, src)

    qkdt = F8 if QKDT == "f8" else BF16
    qT = pqk.tile([128, ST], qkdt, tag="qk")
    kT = pqk.tile([128, ST], qkdt, tag="qk")
    if QKDT == "f8":
        # folded layout for DoubleRow: [32 p, 2 head, 2 dhalf, s]
        qTf = pqk8.tile([32, HPC, 2, ST], F8, tag="qk8")
        kTf = pqk8.tile([32, HPC, 2, ST], F8, tag="qk8")
    # v_aug [128 kpos, 32 gtile, 2 head, 65]; col 64 = ones (from memset)
    v_sb = pv.tile([128, ST // 128, HPC, HD + 1], BF16)
    nc.gpsimd.memset(v_sb[:], WSCALE)
    ones_t = pwo.tile([128, 128], BF16, tag="ones")
    nc.gpsimd.memset(ones_t[:], 1.0)

    # --- projection units (one PSUM tile fill + copy each) --------------
    DR = mybir.MatmulPerfMode.DoubleRow
    PASSES = (("h", "h"), ("h", "l"), ("l", "h"))

    # qT/kT: out[d(128), s] ; lhsT = w [h,d], rhs = x [h,s]; one 512-col half
    def proj_qk(base, w_list, dst, dstf, sbp, half, pp):
        ps = pp.tile([128, 512], F32, tag="pp" if pp is PP else "po",
                     name="psp")
        hs = slice(half * 512, (half + 1) * 512)
        if PROJ == "f8hl":
            seq = [(w8[base, a][ktp][:], x8[b2][ktp][sbp][:, :, hs])
                   for a, b2 in PASSES for ktp in range(NP)]
            if LFT:
                seq.append((w8[base, "L"][:], xL[sbp][:, :, hs]))
            for i, (lh, rh) in enumerate(seq):
                nc.tensor.matmul(ps[:], lh, rh, start=(i == 0),
                                 stop=(i == len(seq) - 1), perf_mode=DR)
        else:
            for kt in range(n_kt):
                nc.tensor.matmul(ps[:], w_list[kt][:],
                                 x_t[kt][sbp][:, hs],
                                 start=(kt == 0), stop=(kt == n_kt - 1))
        cols = slice(sbp * 1024 + half * 512, sbp * 1024 + (half + 1) * 512)
        cp.tensor_copy(dst[:, cols], ps[:])
        if QKDT == "f8":                       # fold quarters into [32,2,2,s]
            for h in range(HPC):
                for dh in range(2):
                    r0 = h * 64 + dh * 32
                    nc.sync.dma_start(dstf[:, h, dh, cols],
                                      dst[r0:r0 + 32, cols])

    # v: out[s(128), (h d)] ; lhsT = x slice [h, s128], rhs = wv
    def proj_v(sbp, half, pp):
        ps = pp.tile([128, 512], F32, tag="pp" if pp is PP else "po",
                     name="psp")
        for ssb in range(4):
            po_ = slice(ssb * 128, (ssb + 1) * 128)
            xo_ = slice(half * 512 + ssb * 128, half * 512 + (ssb + 1) * 128)
            if PROJ == "f8hl":
                seq = [(x8[a][ktp][sbp][:, :, xo_], w8["wv", b2][ktp][:])
                       for a, b2 in PASSES for ktp in range(NP)]
                if LFT:
                    seq.append((xL[sbp][:, :, xo_], w8["wv", "L"][:]))
                for i, (lh, rh) in enumerate(seq):
                    nc.tensor.matmul(ps[:, po_], lh, rh, start=(i == 0),
                                     stop=(i == len(seq) - 1), perf_mode=DR)
            else:
                for kt in range(n_kt):
                    nc.tensor.matmul(ps[:, po_], x_t[kt][sbp][:, xo_],
                                     wv_t[kt][:],
                                     start=(kt == 0), stop=(kt == n_kt - 1))
        # scatter ps cols (g4, h, d) into v slots; col HD stays WSCALE (den)
        pv_in = ps[:].rearrange("p (g h d) -> p g h d", h=HPC, d=HD)
        g0 = sbp * 8 + half * 4
        for h in range(HPC):
            cp.tensor_copy(v_sb[:, g0:g0 + 4, h, 0:HD], pv_in[:, :, h, :])

    def proj_unit(kind, sbp, half, pp):
        if kind == "q":
            proj_qk("wq", None if PROJ == "f8hl" else wq_t, qT,
                    qTf if QKDT == "f8" else None, sbp, half, pp)
        elif kind == "k":
            proj_qk("wk", None if PROJ == "f8hl" else wk_t, kT,
                    kTf if QKDT == "f8" else None, sbp, half, pp)
        else:
            proj_v(sbp, half, pp)

    # minimal projection prologue: only the units ktp0/ctx0 need, so the
    # ACT-bound attention loop starts ~16us earlier.  Everything else
    # streams through the deferred pops (PO psum pool), ordered by first
    # use: k/v units ahead of their consuming ktps, q units ahead of
    # their q-blocks.  (sbp 0-1 = batch 0 tokens, sbp 2-3 = batch 1.)
    for kind, sbp, half in (("q", 0, 0), ("k", 0, 0), ("v", 0, 0)):
        proj_unit(kind, sbp, half, PP)
    deferred = [
        ("k", 0, 1), ("v", 0, 1), ("k", 1, 0), ("v", 1, 0),
        ("k", 1, 1), ("v", 1, 1), ("q", 0, 1), ("q", 1, 0),
        ("q", 1, 1), ("k", 2, 0), ("v", 2, 0), ("k", 2, 1),
        ("v", 2, 1), ("k", 3, 0), ("v", 3, 0), ("q", 2, 0),
        ("k", 3, 1), ("v", 3, 1), ("q", 2, 1), ("q", 3, 0), ("q", 3, 1),
    ]

    # --- attention + out-projection, per (batch, q-block) ---------------
    def finalize(q0, cps):
        # normalize: rec = 1/den (bf16, row 64); broadcast via ones-matmul
        rden = prec.tile([128, 2, 512], BF16, tag="rden")
        rbs = prec.tile([128, 2, 512], BF16, tag="rbs")
        cq = pcq.tile([128, 512], BF16, tag="cq")
        c1 = pc1.tile([128, 512], BF16, tag="c1")
        for h in range(2):
            with nc.allow_low_precision(reason="bf16 recip feeds matmul"):
                nc.vector.reciprocal(rden[64:65, h, :], cps[h][64:65, :])
            rbh = PO.tile([128, 512], F32, tag="po", name="rb")
            nc.tensor.matmul(rbh[:], ones_t[64:65, :], rden[64:65, h, :],
                             start=True, stop=True)
            cp.tensor_copy(rbs[0:64, h, :], rbh[0:64, :])
        nc.vector.tensor_mul(cq[0:64, :], cps[0][0:64, :], rbs[0:64, 0, :])
        nc.vector.tensor_mul(c1[0:64, :], cps[1][0:64, :], rbs[0:64, 1, :])
        nc.sync.dma_start(cq[64:128, :], c1[0:64, :])
        return cq

    def finalize_proj(q0, cq, ssbs):
        # out rows q0..q0+512 : lhsT = cq col-slice, rhs = wo
        for ssb in ssbs:
            c0 = q0 + ssb * 128
            ot = pout.tile([128, H], odt, tag="ot")
            for e in range(2):
                po = PO.tile([128, 512], F32, tag="po")
                nc.tensor.matmul(po[:], cq[:, ssb * 128:(ssb + 1) * 128],
                                 wo_t[:, e * 512:(e + 1) * 512],
                                 start=True, stop=True)
                cp.tensor_copy(ot[:, e * 512:(e + 1) * 512], po[:])
            nc.sync.dma_start(out[c0:c0 + 128, :], ot[:])

    def emit_ctx(cps, b, ktp, expt):
        for j in range(2):                     # ctx+den accumulate
            kt = ktp * 2 + j
            g = b * NKT_S + kt
            st = (ktp == 0 and j == 0)
            sp = (ktp == NKT_S // 2 - 1 and j == 1)
            for h in range(2):
                nc.tensor.matmul(cps[h][:, :],
                                 v_sb[:, g, h, :],
                                 expt[h][:, j * 512:(j + 1) * 512],
                                 start=st, stop=sp)

    pending = None                             # (q0, cps) awaiting finalize
    lag = None                                 # (cps, b, ktp, expt) ctx lag
    for b in range(B):
        for qb in range(NQB):
            q0 = b * S + qb * 512              # global column of this q-block
            # per-head psum bank: ctx rows 0-63, den row 64
            cps = [PC.tile([65, 512], F32, tag="pc", name=f"cps{h}")
                   for h in range(2)]
            for ktp in range(NKT_S // 2):
                expt = []
                for h in range(2):             # scores, row-packed pairs
                    sc = PP.tile([128, 1024], F32, tag="pp")
                    for j in range(2):
                        kt = ktp * 2 + j
                        k0 = b * S + kt * 128
                        if QKDT == "f8":
                            nc.tensor.matmul(
                                sc[:, j * 512:(j + 1) * 512],
                                kTf[:, h, :, k0:k0 + 128],
                                qTf[:, h, :, q0:q0 + 512],
                                start=True, stop=True,
                                perf_mode=mybir.MatmulPerfMode.DoubleRow)
                        else:
                            nc.tensor.matmul(
                                sc[:, j * 512:(j + 1) * 512],
                                kT[h * 64:(h + 1) * 64, k0:k0 + 128],
                                qT[h * 64:(h + 1) * 64, q0:q0 + 512],
                                start=True, stop=True)
                    e = pexp.tile([128, 1024], BF16)
                    nc.scalar.activation(e[:], sc[:],
                                         mybir.ActivationFunctionType.Exp,
                                         scale=0.125 / (WSCALE * WSCALE))
                    expt.append(e)
                # ctx lags one ktp behind scores/exp so PE never waits on
                # ACT; the lagged last-ktp ctx spills past the next block's
                # first scores, closing the block-boundary gap
                if lag is not None:
                    emit_ctx(*lag)
                lag = (cps, b, ktp, expt)
                if ktp == 0 and pending is not None:
                    pending_cq = finalize(*pending)
                elif ktp == 1 and pending is not None:
                    finalize_proj(pending[0], pending_cq, (0, 1))
                elif ktp == 2 and pending is not None:
                    finalize_proj(pending[0], pending_cq, (2, 3))
                    pending = None
                elif deferred:
                    proj_unit(*deferred.pop(0), PO)
            pending = (q0, cps)
    emit_ctx(*lag)
    finalize_proj(pending[0], finalize(*pending), (0, 1, 2, 3))


_CACHE = {}


def _get_nc(n_kt):
    if n_kt not in _CACHE:
        _CACHE[n_kt] = _build(n_kt)
    return _CACHE[n_kt]


def _prep_inputs(hidden_states, Wq, bq, Wk, bk, Wv, bv, Wo, bo):
    x = np.ascontiguousarray(np.asarray(hidden_states, np.float32)
                             .reshape(ST, H))
    bias = not (np.all(bq == 0) and np.all(bk == 0) and np.all(bv == 0))
    n_kt = 9 if bias else 8
    xTn = np.zeros((n_kt * 128, ST), np.float32)
    xTn[:H] = x.T
    if bias:
        xTn[H] = 1.0
    
    if PROJ == "f8hl":
        xhi = xTn.astype(NPF8)
        xm = {"xhi": xhi,
              "xlo": (xTn - xhi.astype(np.float32)).astype(NPF8)}
    else:
        xm = {"xT": xTn.astype(NPBF16)}

    in_maps = []
    for c in range(NCORES):
        rows = slice(c * DSH, (c + 1) * DSH)
        m = dict(xm)
        for name, W, bvec in (("wq", Wq, bq), ("wk", Wk, bk), ("wv", Wv, bv)):
            wt = np.zeros((n_kt * 128, DSH), np.float32)
            wt[:H] = np.asarray(W, np.float32)[rows, :].T
            if bias:
                wt[H] = np.asarray(bvec, np.float32)[rows]
            wt *= WSCALE
            if PROJ == "f8hl":
                whi = wt.astype(NPF8)
                m[name + "hi"] = whi
                m[name + "lo"] = (wt - whi.astype(np.float32)).astype(NPF8)
            else:
                m[name] = wt.astype(NPBF16)
        m["wo"] = np.ascontiguousarray(
            np.asarray(Wo, np.float32)[:, rows].T).astype(NPBF16)
        in_maps.append(m)
    return n_kt, in_maps


def kernel(hidden_states, Wq, bq, Wk, bk, Wv, bv, Wo, bo, _return_extras=False):
    n_kt, in_maps = _prep_inputs(hidden_states, Wq, bq, Wk, bk, Wv, bv, Wo, bo)
    nc = _get_nc(n_kt)
    res = bass_utils.run_bass_kernel_spmd(nc, in_maps,
                                          core_ids=list(range(NCORES)))
    acc = res.results[0]["out"].astype(np.float64)
    for c in range(1, NCORES):
        acc += res.results[c]["out"]
    acc += np.asarray(bo, np.float64)
    outv = acc.astype(np.float32).reshape(B, S, H)
    if _return_extras:
        return outv, (nc, in_maps, res)
    return outv

